# revision 8
# baseline (speedup 1.0000x reference)
"""Trainium2 Bass kernel for a 16-head MHA layer (batch 4, seq 2048, embed 1024).

Sharding: 8 cores; core c handles batch c//2 and query-token half c%2.
Each core receives its batch's x rotated so that its 1024 query tokens sit in
rows 0:1024 (softmax/attention is permutation-invariant over key order, so the
rotation changes nothing mathematically). K/V are computed over the full
sequence on-core, so no collectives are needed. Weights are replicated.

Compute structure per core:
 - QKV projections in bf16 (fp32 PSUM accumulation).
 - Scores run as fp8(e4m3) DoubleRow matmuls: K and Q are staged in fp8 with
   each head's 64 d-dims folded to [32 partitions, 2 column-halves] so one
   matmul contracts all 64 dims at 0.5 cycles/row.
 - AV runs transposed: P (exp of scores) is the stationary operand and V
   streams 64+1 columns (the extra ones-column produces the softmax
   denominator in the same matmul). Output lands as [q, d] in PSUM.
 - Per (head, q-tile): reciprocal of the denominator + fused scale on DVE,
   then a DMA-transpose flips the pair back to [d, q] for the out-proj.
 - V-bias folds through softmax (sum(attn)=1) into the out-proj bias.
"""

import sys

for _p in ("/opt/trn_rl_repo",):
    if _p not in sys.path:
        sys.path.insert(0, _p)

import numpy as np

import concourse.bass as bass  # noqa: E402
import concourse.mybir as mybir  # noqa: E402
import concourse.tile as tile  # noqa: E402
from concourse import bacc  # noqa: E402

SEQ = 2048
E = 1024
H = 16
D = 64
NQ = 1024  # query tokens per core
N_CORES = 8

ET = E // 128   # 8 e-chunks
TT = SEQ // 128  # 16 token tiles
KT = SEQ // 128  # 16 key tiles
QT = NQ // 128   # 8 query tiles

F32 = mybir.dt.float32
BF16 = mybir.dt.bfloat16
F8 = mybir.dt.float8e4
AF = mybir.ActivationFunctionType
DR = mybir.MatmulPerfMode.DoubleRow


def build_program():
    nc = bacc.Bacc(trn_type="TRN2", target_bir_lowering=False, debug=False)

    x = nc.dram_tensor("x", [SEQ, E], F32, kind="ExternalInput").ap()
    wqkv = nc.dram_tensor("Wqkv", [E, 3 * E], F32, kind="ExternalInput").ap()
    bqkv = nc.dram_tensor("bqkv", [3 * E], F32, kind="ExternalInput").ap()
    wo = nc.dram_tensor("Wo", [E, E], F32, kind="ExternalInput").ap()
    bo = nc.dram_tensor("bo", [E], F32, kind="ExternalInput").ap()
    out = nc.dram_tensor("out", [NQ, E], F32, kind="ExternalOutput").ap()

    with tile.TileContext(nc) as tc:
        _body(nc, tc, x, wqkv, bqkv, wo, bo, out)

    nc.compile()
    return nc


def _body(nc, tc, x, wqkv, bqkv, wo, bo, out):
    from contextlib import ExitStack

    es = ExitStack()
    with es:
        pc = es.enter_context(tc.tile_pool(name="const", bufs=1))
        pbig = es.enter_context(tc.tile_pool(name="big", bufs=1))
        pxb = es.enter_context(tc.tile_pool(name="xb", bufs=4))
        pskin = es.enter_context(tc.tile_pool(name="skin", bufs=2))
        pprt = es.enter_context(tc.tile_pool(name="part", bufs=1))
        pkstg = es.enter_context(tc.tile_pool(name="kstg", bufs=2))
        pqstg = es.enter_context(tc.tile_pool(name="qstg", bufs=2))
        pw = es.enter_context(tc.tile_pool(name="wpan", bufs=3))
        ppt = es.enter_context(tc.tile_pool(name="pt", bufs=8))
        pasb = es.enter_context(tc.tile_pool(name="asb", bufs=8))
        prec = es.enter_context(tc.tile_pool(name="rec", bufs=4))
        posb = es.enter_context(tc.tile_pool(name="osb", bufs=2))
        pps = es.enter_context(tc.tile_pool(name="ps_s", bufs=2, space="PSUM"))
        pav = es.enter_context(tc.tile_pool(name="ps_av", bufs=2, space="PSUM"))
        ppj = es.enter_context(tc.tile_pool(name="ps_pj", bufs=2, space="PSUM"))

        # --- persistent SBUF tensors -----------------------------------
        xT = pbig.tile([128, ET, SEQ], BF16, tag="xT")
        # fp8 DoubleRow K/Q: tile g holds heads 4g..4g+3 at partition blocks
        # 32a; free dims = [d-half, token]
        K8 = [pbig.tile([128, 2, SEQ], F8, tag=f"k8_{g}", name=f"k8_{g}") for g in range(4)]
        Q8 = [pbig.tile([128, 2, NQ], F8, tag=f"q8_{g}", name=f"q8_{g}") for g in range(4)]
        # V for the AV matmul: [token-part, kt, head, 65]; col 64 = ones
        VO = pbig.tile([128, KT, H, 65], BF16, tag="vo")
        # attention output transposed: AT[hp] rows = heads 2hp,2hp+1 (64 d each)
        AT = [pbig.tile([128, NQ], BF16, tag=f"at{p}", name=f"at{p}") for p in range(H // 2)]

        nc.vector.memset(VO[:, :, :, 64:65], 1.0)

        # biases: bqkvT[p, c] = bqkv[128c + p] via strided DRAM read
        bqkvT = pc.tile([128, 24], F32, tag="bqkvT")
        nc.gpsimd.dma_start(out=bqkvT, in_=bqkv.rearrange("(c p) -> p c", p=128))
        boT = pc.tile([128, E], F32, tag="boT")
        bo_bcast = bass.AP(tensor=bo.tensor, offset=bo.offset, ap=[[0, 128]] + bo.ap)
        nc.gpsimd.dma_start(out=boT, in_=bo_bcast)
        boB = pc.tile([128, E], F32, tag="boB")

        # --- weight panels: ONE gpsimd DMA each (casts f32 -> bf16) ----
        def load_panel(pc0, src_w=None):
            src_w = wqkv if src_w is None else src_w
            wp = pw.tile([128, ET, 512], BF16, tag="wp", name=f"wp{id(src_w)}_{pc0}")
            nc.gpsimd.dma_start(
                out=wp,
                in_=src_w[:, pc0 : pc0 + 512].rearrange("(ee p) c -> p ee c", p=128),
            )
            return wp

        def load_skinny(pc0):
            # [128, ET, 128] panel for one pair-tile, one cast DMA
            ws = pskin.tile([128, ET, 128], BF16, tag="wskin", name=f"wsk{pc0}")
            nc.gpsimd.dma_start(
                out=ws,
                in_=wqkv[:, pc0 : pc0 + 128].rearrange("(ee p) c -> p ee c", p=128),
            )
            return ws

        # --- x load (gpsimd cast DMA) + DMA-transpose --------------------
        def load_x(tt):
            xb = pxb.tile([128, E], BF16, tag="xb", name=f"xb{tt}")
            nc.gpsimd.dma_start(out=xb, in_=x[tt * 128 : (tt + 1) * 128, :])
            nc.scalar.dma_start(
                out=xT[:, :, tt * 128 : (tt + 1) * 128], in_=xb, transpose=True
            )

        # --- projection chains -----------------------------------------
        def kq_chain(wp, kind, t, tb, stage, skinny=False):
            # one K^T/Q^T proj tile [128, 512 tokens] for pair-tile t,
            # written with bias into the fp8 stage tile.
            bcol = ((E if kind == "k" else 0) + t * 128) // 128
            ps = ppj.tile([128, 512], F32, tag="ps")
            for ee in range(ET):
                lhsT = wp[:, ee, :] if skinny else wp[:, ee, (t % 4) * 128 : (t % 4 + 1) * 128]
                nc.tensor.matmul(
                    ps,
                    lhsT=lhsT,
                    rhs=xT[:, ee, tb * 512 : (tb + 1) * 512],
                    start=(ee == 0),
                    stop=(ee == ET - 1),
                )
            nc.vector.tensor_scalar_add(
                stage[:, tb * 512 : (tb + 1) * 512], ps, bqkvT[:, bcol : bcol + 1]
            )

        def kq_folds(kind, t, stage, tb=None):
            # stage [128, ntok] fp8 -> K8/Q8[g] partition fold (4 DMAs);
            # tb=None folds the whole token range, else just that 512-block
            g = t // 2
            dst = K8[g] if kind == "k" else Q8[g]
            a0 = 2 * (t % 2)
            c0, c1 = (0, dst.shape[-1]) if tb is None else (tb * 512, (tb + 1) * 512)
            for s in range(4):
                nc.sync.dma_start(
                    out=dst[32 * (a0 + s // 2) : 32 * (a0 + s // 2) + 32, s % 2, c0:c1],
                    in_=stage[32 * s : 32 * s + 32, c0:c1],
                )

        def k_tile(wp, t):
            stage = pkstg.tile([128, SEQ], F8, tag="kstg", name=f"kstg{t}")
            for tb in range(4):
                kq_chain(wp, "k", t, tb, stage)
            kq_folds("k", t, stage)

        def q_tile(wp, t):
            stage = pqstg.tile([128, NQ], F8, tag="qstg", name=f"qstg{t}")
            for tb in range(2):
                kq_chain(wp, "q", t, tb, stage)
            kq_folds("q", t, stage)

        def v_chain(wp, panel, tt):
            # V proj tile [128 tok, 512 w-cols] -> VO[:, tt, 8p:8p+8, 0:64]
            ps = ppj.tile([128, 512], F32, tag="ps")
            for ee in range(ET):
                nc.tensor.matmul(
                    ps,
                    lhsT=xT[:, ee, tt * 128 : (tt + 1) * 128],
                    rhs=wp[:, ee, :],
                    start=(ee == 0),
                    stop=(ee == ET - 1),
                )
            nc.vector.tensor_copy(
                VO[:, tt, 8 * panel : 8 * panel + 8, 0:64],
                ps.rearrange("p (h d) -> p h d", d=64),
            )

        # --- out-proj ---------------------------------------------------
        wob = [None, None]

        def boB_setup():
            # attn-out = AV/den + bv  (V-bias passes softmax unchanged), so
            # out = A_nobias @ Wo + (bv @ Wo + bo) = A_nobias @ Wo + boB
            ones128 = pc.tile([128, 128], BF16, tag="ones128")
            nc.vector.memset(ones128, 1.0)
            bv_rep = pc.tile([128, ET, 128], BF16, tag="bvrep")
            for ee in range(ET):
                nc.vector.tensor_scalar_mul(
                    bv_rep[:, ee, :], ones128, bqkvT[:, 16 + ee : 17 + ee]
                )
            for half in range(2):
                c0 = half * 512
                psb = ppj.tile([128, 512], F32, tag="ps")
                for ee in range(ET):
                    nc.tensor.matmul(
                        psb,
                        lhsT=bv_rep[:, ee, :],
                        rhs=wob[half][:, ee, :],
                        start=(ee == 0),
                        stop=(ee == ET - 1),
                    )
                nc.vector.tensor_add(boB[:, c0 : c0 + 512], psb, boT[:, c0 : c0 + 512])

        # three-stage out-proj: pass A (e-chunks 0:6 = pairs 0-5, windows
        # 12-14) -> bf16 partials incl. boB; pass B1 (chunk 6, windows
        # 14-15) adds pair 6; pass B2 (chunk 7) is the only tail work.
        NEA = 6
        partial = pprt.tile([128, 16, 512], BF16, tag="partial")

        def outproj_passA(tt, half):
            c0 = half * 512
            ps = ppj.tile([128, 512], F32, tag="ps")
            for ee in range(NEA):
                nc.tensor.matmul(
                    ps,
                    lhsT=AT[ee][:, tt * 128 : (tt + 1) * 128],
                    rhs=wob[half][:, ee, :],
                    start=(ee == 0),
                    stop=(ee == NEA - 1),
                )
            nc.vector.tensor_add(
                partial[:, tt * 2 + half, :], ps, boB[:, c0 : c0 + 512]
            )

        def outproj_passB1(tt, half):
            ps = ppj.tile([128, 512], F32, tag="ps")
            nc.tensor.matmul(
                ps,
                lhsT=AT[NEA][:, tt * 128 : (tt + 1) * 128],
                rhs=wob[half][:, NEA, :],
                start=True,
                stop=True,
            )
            nc.vector.scalar_tensor_tensor(
                partial[:, tt * 2 + half, :], ps, 1.0,
                partial[:, tt * 2 + half, :],
                op0=mybir.AluOpType.mult, op1=mybir.AluOpType.add,
            )

        def outproj_passB2(tt, half):
            c0 = half * 512
            ps = ppj.tile([128, 512], F32, tag="ps")
            nc.tensor.matmul(
                ps,
                lhsT=AT[7][:, tt * 128 : (tt + 1) * 128],
                rhs=wob[half][:, 7, :],
                start=True,
                stop=True,
            )
            osb = posb.tile([128, 512], F32, tag="osb")
            nc.vector.tensor_add(osb, ps, partial[:, tt * 2 + half, :])
            nc.sync.dma_start(
                out=out[tt * 128 : (tt + 1) * 128, c0 : c0 + 512], in_=osb
            )

        # --- prologue ---------------------------------------------------
        # PE warmup: keep the tensor engine busy from t~1us so the p-state
        # ramp completes before the first real chain arrives.
        warm = pc.tile([128, 512], BF16, tag="warm")
        nc.vector.memset(warm, 0.001)
        ps_w = pps.tile([128, NQ], F32, tag="ps_s", name="warmps")
        for i in range(26):
            nc.tensor.matmul(
                ps_w[:, 0:512], lhsT=warm[:, 0:128], rhs=warm,
                start=True, stop=True, skip_group_check=True,
            )

        ws_q0 = load_skinny(0)
        ws_k0 = load_skinny(E)
        for tt in range(16):
            load_x(tt)
        wp_k0 = load_panel(E)
        wp_q0 = load_panel(0)
        wp_v0 = load_panel(2 * E)
        wp_v1_pro = load_panel(2 * E + 512)

        # pair-tile 0 with per-tb folds, Q before K so full-Q is ready first
        stg_q0 = pqstg.tile([128, NQ], F8, tag="qstg", name="qstg0")
        stg_k0 = pkstg.tile([128, SEQ], F8, tag="kstg", name="kstg0")
        for tb in range(2):
            kq_chain(ws_q0, "q", 0, tb, stg_q0, skinny=True)
            kq_folds("q", 0, stg_q0, tb=tb)
        for tb in range(4):
            kq_chain(ws_k0, "k", 0, tb, stg_k0, skinny=True)
            kq_folds("k", 0, stg_k0, tb=tb)

        # --- deferred proj work, paced one job per kt slot ---------------
        wp_box = {}

        def defer_panel(name, pc0, src=None):
            def job():
                wp_box[name] = load_panel(pc0, src_w=src)

            return job

        def kq_jobs(kind, wp_name, t):
            # one chain per job; folds ride with the last chain
            ntb = 4 if kind == "k" else 2
            stage_box = {}

            def mk(tb):
                def job():
                    if tb == 0:
                        pool, shape = (pkstg, SEQ) if kind == "k" else (pqstg, NQ)
                        stage_box[0] = pool.tile(
                            [128, shape], F8, tag="kstg" if kind == "k" else "qstg",
                            name=f"{kind}stg{t}",
                        )
                    wp = wp_box[wp_name] if isinstance(wp_name, str) else wp_name
                    kq_chain(wp, kind, t, tb, stage_box[0])
                    if tb == ntb - 1:
                        kq_folds(kind, t, stage_box[0])

                return job

            return [mk(tb) for tb in range(ntb)]

        v1j = [(lambda t=t: v_chain(wp_v1_pro, 1, t)) for t in range(16)]
        jobs = {
            1: kq_jobs("k", wp_k0, 1) + kq_jobs("q", wp_q0, 1),
            2: v1j[0:5] + [defer_panel("k1", E + 512)],
            3: kq_jobs("k", wp_k0, 2) + kq_jobs("q", wp_q0, 2),
            4: v1j[5:10] + [defer_panel("q1", 512)],
            5: kq_jobs("k", wp_k0, 3) + kq_jobs("q", wp_q0, 3),
            6: v1j[10:16],
            7: kq_jobs("k", "k1", 4) + kq_jobs("q", "q1", 4),
            8: kq_jobs("k", "k1", 5) + kq_jobs("q", "q1", 5)
               + [defer_panel("wo0", 0, wo), defer_panel("wo1", 512, wo)],
            9: kq_jobs("k", "k1", 6) + kq_jobs("q", "q1", 6),
            10: kq_jobs("k", "k1", 7) + kq_jobs("q", "q1", 7),
            11: [lambda: (wob.__setitem__(0, wp_box["wo0"]),
                          wob.__setitem__(1, wp_box["wo1"])),
                 boB_setup],
            12: [(lambda t=t: outproj_passA(t, 0)) for t in range(6)],
            13: [(lambda t=t: outproj_passA(t, 1)) for t in range(6)]
               + [(lambda t=t: outproj_passA(t, 0)) for t in range(6, 8)],
            14: [(lambda t=t: outproj_passA(t, 1)) for t in range(6, 8)]
               + [(lambda t=t: outproj_passB1(t, 0)) for t in range(8)],
            15: [(lambda t=t: outproj_passB1(t, 1)) for t in range(8)],
        }

        # --- attention --------------------------------------------------
        cur_asb = [None] * QT

        def attention_head(h, inner=None):
            g, a = h // 4, h % 4
            p0 = 32 * a
            av = [
                pav.tile([128, 4, 65], F32, tag="av", name=f"av{h}_{i}")
                for i in range(2)
            ]
            for kt in range(KT):
                if inner is not None and kt < len(inner):
                    inner[kt]()
                ps_s = pps.tile([128, NQ], F32, tag="ps_s")
                for qh in range(2):
                    nc.tensor.matmul(
                        ps_s[:, qh * 512 : (qh + 1) * 512],
                        lhsT=K8[g][p0 : p0 + 32, :, kt * 128 : (kt + 1) * 128],
                        rhs=Q8[g][p0 : p0 + 32, :, qh * 512 : (qh + 1) * 512],
                        start=True,
                        stop=True,
                        perf_mode=DR,
                        tile_position=(p0, 0),
                    )
                pt = ppt.tile([128, NQ], BF16, tag="pt")
                nc.scalar.activation(pt, ps_s, AF.Exp, scale=0.125)
                for qt in range(QT):
                    nc.tensor.matmul(
                        av[qt // 4][:, qt % 4, :],
                        lhsT=pt[:, qt * 128 : (qt + 1) * 128],
                        rhs=VO[:, kt, h, :],
                        start=(kt == 0 and qt % 4 == 0),
                        stop=(kt == KT - 1),
                        skip_group_check=True,
                    )
            # normalize into a_sb (col half h%2); DMA-transpose pairs into AT
            hp = h // 2
            for qt in range(QT):
                if h % 2 == 0:
                    cur_asb[qt] = pasb.tile(
                        [128, 128], BF16, tag="asb", name=f"asb{h}_{qt}"
                    )
                rec = prec.tile([128, 1], F32, tag="rec", name=f"rec{h}_{qt}")
                nc.vector.reciprocal_approx_fast(
                    rec, av[qt // 4][:, qt % 4, 64:65]
                )
                nc.vector.tensor_scalar_mul(
                    cur_asb[qt][:, 64 * (h % 2) : 64 * (h % 2) + 64],
                    av[qt // 4][:, qt % 4, 0:64],
                    rec,
                )
                if h % 2 == 1:
                    nc.sync.dma_start(
                        out=AT[hp][:, qt * 128 : (qt + 1) * 128],
                        in_=cur_asb[qt],
                        transpose=True,
                    )

        inner0 = [(lambda t=tt: v_chain(wp_v0, 0, t)) for tt in range(TT)]
        for h in range(H):
            if h == 0:
                attention_head(h, inner=inner0)
                continue
            window_jobs = jobs.get(h, [])
            inner = [(lambda j=j: j()) for j in window_jobs]
            attention_head(h, inner=inner)

        # --- out-proj tail (pass B2 only) -------------------------------
        for tt in range(8):
            for half in range(2):
                outproj_passB2(tt, half)


_NC = None


def _get_program():
    global _NC
    if _NC is None:
        _NC = build_program()
    return _NC


def make_in_maps(x, Wqkv, bqkv, Wo, bo):
    w = {
        "Wqkv": np.ascontiguousarray(np.asarray(Wqkv, np.float32)),
        "bqkv": np.ascontiguousarray(np.asarray(bqkv, np.float32)),
        "Wo": np.ascontiguousarray(np.asarray(Wo, np.float32)),
        "bo": np.ascontiguousarray(np.asarray(bo, np.float32)),
    }
    x = np.asarray(x, np.float32)
    in_maps = []
    for c in range(N_CORES):
        b, s = divmod(c, 2)
        xb = x[b]
        if s == 1:
            xb = np.roll(xb, -NQ, axis=0)
        in_maps.append({"x": np.ascontiguousarray(xb), **w})
    return in_maps


def gather_out(results):
    out = np.empty((4, SEQ, E), np.float32)
    for c in range(N_CORES):
        b, s = divmod(c, 2)
        out[b, s * NQ : (s + 1) * NQ] = results[c]["out"]
    return out


def kernel(x, Wqkv, bqkv, Wo, bo):
    from concourse.bass_utils import run_bass_kernel_spmd

    nc = _get_program()
    in_maps = make_in_maps(x, Wqkv, bqkv, Wo, bo)
    res = run_bass_kernel_spmd(nc, in_maps, core_ids=list(range(N_CORES)))
    return gather_out(res.results)


# revision 9
# speedup vs baseline: 1.0391x; 1.0391x over previous
"""Trainium2 Bass kernel for a 16-head MHA layer (batch 4, seq 2048, embed 1024).

Sharding: 8 cores; core c handles batch c//2 and query-token half c%2.
Each core receives its batch's x rotated so that its 1024 query tokens sit in
rows 0:1024 (softmax/attention is permutation-invariant over key order, so the
rotation changes nothing mathematically). K/V are computed over the full
sequence on-core, so no collectives are needed. Weights are replicated.

Compute structure per core:
 - QKV projections in bf16 (fp32 PSUM accumulation).
 - Scores run as fp8(e4m3) DoubleRow matmuls: K and Q are staged in fp8 with
   each head's 64 d-dims folded to [32 partitions, 2 column-halves] so one
   matmul contracts all 64 dims at 0.5 cycles/row.
 - AV runs transposed: P (exp of scores) is the stationary operand and V
   streams 64+1 columns (the extra ones-column produces the softmax
   denominator in the same matmul). Output lands as [q, d] in PSUM.
 - Per (head, q-tile): reciprocal of the denominator + fused scale on DVE,
   then a DMA-transpose flips the pair back to [d, q] for the out-proj.
 - V-bias folds through softmax (sum(attn)=1) into the out-proj bias.
"""

import sys

for _p in ("/opt/trn_rl_repo",):
    if _p not in sys.path:
        sys.path.insert(0, _p)

import numpy as np

import concourse.bass as bass  # noqa: E402
import concourse.mybir as mybir  # noqa: E402
import concourse.tile as tile  # noqa: E402
from concourse import bacc  # noqa: E402

SEQ = 2048
E = 1024
H = 16
D = 64
NQ = 1024  # query tokens per core
N_CORES = 8

ET = E // 128   # 8 e-chunks
TT = SEQ // 128  # 16 token tiles
KT = SEQ // 128  # 16 key tiles
QT = NQ // 128   # 8 query tiles

F32 = mybir.dt.float32
BF16 = mybir.dt.bfloat16
F8 = mybir.dt.float8e4
AF = mybir.ActivationFunctionType
DR = mybir.MatmulPerfMode.DoubleRow


def build_program():
    nc = bacc.Bacc(trn_type="TRN2", target_bir_lowering=False, debug=False)

    x = nc.dram_tensor("x", [SEQ, E], F32, kind="ExternalInput").ap()
    wqkv = nc.dram_tensor("Wqkv", [E, 3 * E], F32, kind="ExternalInput").ap()
    bqkv = nc.dram_tensor("bqkv", [3 * E], F32, kind="ExternalInput").ap()
    wo = nc.dram_tensor("Wo", [E, E], F32, kind="ExternalInput").ap()
    bo = nc.dram_tensor("bo", [E], F32, kind="ExternalInput").ap()
    out = nc.dram_tensor("out", [NQ, E], F32, kind="ExternalOutput").ap()

    with tile.TileContext(nc) as tc:
        _body(nc, tc, x, wqkv, bqkv, wo, bo, out)

    nc.compile()
    return nc


def _body(nc, tc, x, wqkv, bqkv, wo, bo, out):
    from contextlib import ExitStack

    es = ExitStack()
    with es:
        pc = es.enter_context(tc.tile_pool(name="const", bufs=1))
        pbig = es.enter_context(tc.tile_pool(name="big", bufs=1))
        pxb = es.enter_context(tc.tile_pool(name="xb", bufs=4))
        pskin = es.enter_context(tc.tile_pool(name="skin", bufs=2))
        pprt = es.enter_context(tc.tile_pool(name="part", bufs=1))
        pkstg = es.enter_context(tc.tile_pool(name="kstg", bufs=2))
        pqstg = es.enter_context(tc.tile_pool(name="qstg", bufs=2))
        pw = es.enter_context(tc.tile_pool(name="wpan", bufs=3))
        ppt = es.enter_context(tc.tile_pool(name="pt", bufs=8))
        pasb = es.enter_context(tc.tile_pool(name="asb", bufs=8))
        prec = es.enter_context(tc.tile_pool(name="rec", bufs=4))
        posb = es.enter_context(tc.tile_pool(name="osb", bufs=2))
        pps = es.enter_context(tc.tile_pool(name="ps_s", bufs=2, space="PSUM"))
        pav = es.enter_context(tc.tile_pool(name="ps_av", bufs=2, space="PSUM"))
        ppj = es.enter_context(tc.tile_pool(name="ps_pj", bufs=2, space="PSUM"))

        # --- persistent SBUF tensors -----------------------------------
        xT = pbig.tile([128, ET, SEQ], BF16, tag="xT")
        # fp8 DoubleRow K/Q: tile g holds heads 4g..4g+3 at partition blocks
        # 32a; free dims = [d-half, token]
        K8 = [pbig.tile([128, 2, SEQ], F8, tag=f"k8_{g}", name=f"k8_{g}") for g in range(4)]
        Q8 = [pbig.tile([128, 2, NQ], F8, tag=f"q8_{g}", name=f"q8_{g}") for g in range(4)]
        # V for the AV matmul: [token-part, kt, head, 65]; col 64 = ones
        VO = pbig.tile([128, KT, H, 65], BF16, tag="vo")
        # attention output transposed: AT[hp] rows = heads 2hp,2hp+1 (64 d each)
        AT = [pbig.tile([128, NQ], BF16, tag=f"at{p}", name=f"at{p}") for p in range(H // 2)]

        nc.vector.memset(VO[:, :, :, 64:65], 1.0)

        # biases: bqkvT[p, c] = bqkv[128c + p] via strided DRAM read
        bqkvT = pc.tile([128, 24], F32, tag="bqkvT")
        nc.gpsimd.dma_start(out=bqkvT, in_=bqkv.rearrange("(c p) -> p c", p=128))
        boT = pc.tile([128, E], F32, tag="boT")
        bo_bcast = bass.AP(tensor=bo.tensor, offset=bo.offset, ap=[[0, 128]] + bo.ap)
        nc.gpsimd.dma_start(out=boT, in_=bo_bcast)
        boB = pc.tile([128, E], F32, tag="boB")

        # --- weight panels: ONE gpsimd DMA each (casts f32 -> bf16) ----
        def load_panel(pc0, src_w=None):
            src_w = wqkv if src_w is None else src_w
            wp = pw.tile([128, ET, 512], BF16, tag="wp", name=f"wp{id(src_w)}_{pc0}")
            nc.gpsimd.dma_start(
                out=wp,
                in_=src_w[:, pc0 : pc0 + 512].rearrange("(ee p) c -> p ee c", p=128),
            )
            return wp

        def load_skinny(pc0):
            # [128, ET, 128] panel for one pair-tile, one cast DMA
            ws = pskin.tile([128, ET, 128], BF16, tag="wskin", name=f"wsk{pc0}")
            nc.gpsimd.dma_start(
                out=ws,
                in_=wqkv[:, pc0 : pc0 + 128].rearrange("(ee p) c -> p ee c", p=128),
            )
            return ws

        # --- x load (gpsimd cast DMA) + DMA-transpose --------------------
        def load_x(tt):
            xb = pxb.tile([128, E], BF16, tag="xb", name=f"xb{tt}")
            nc.gpsimd.dma_start(out=xb, in_=x[tt * 128 : (tt + 1) * 128, :])
            nc.scalar.dma_start(
                out=xT[:, :, tt * 128 : (tt + 1) * 128], in_=xb, transpose=True
            )

        # --- projection chains -----------------------------------------
        def kq_chain(wp, kind, t, tb, stage, skinny=False):
            # one K^T/Q^T proj tile [128, 512 tokens] for pair-tile t,
            # written with bias into the fp8 stage tile.
            bcol = ((E if kind == "k" else 0) + t * 128) // 128
            ps = ppj.tile([128, 512], F32, tag="ps")
            for ee in range(ET):
                lhsT = wp[:, ee, :] if skinny else wp[:, ee, (t % 4) * 128 : (t % 4 + 1) * 128]
                nc.tensor.matmul(
                    ps,
                    lhsT=lhsT,
                    rhs=xT[:, ee, tb * 512 : (tb + 1) * 512],
                    start=(ee == 0),
                    stop=(ee == ET - 1),
                )
            nc.vector.tensor_scalar_add(
                stage[:, tb * 512 : (tb + 1) * 512], ps, bqkvT[:, bcol : bcol + 1]
            )

        def kq_folds(kind, t, stage, tb=None):
            # stage [128, ntok] fp8 -> K8/Q8[g] partition fold (4 DMAs);
            # tb=None folds the whole token range, else just that 512-block
            g = t // 2
            dst = K8[g] if kind == "k" else Q8[g]
            a0 = 2 * (t % 2)
            c0, c1 = (0, dst.shape[-1]) if tb is None else (tb * 512, (tb + 1) * 512)
            for s in range(4):
                nc.sync.dma_start(
                    out=dst[32 * (a0 + s // 2) : 32 * (a0 + s // 2) + 32, s % 2, c0:c1],
                    in_=stage[32 * s : 32 * s + 32, c0:c1],
                )

        def k_tile(wp, t):
            stage = pkstg.tile([128, SEQ], F8, tag="kstg", name=f"kstg{t}")
            for tb in range(4):
                kq_chain(wp, "k", t, tb, stage)
            kq_folds("k", t, stage)

        def q_tile(wp, t):
            stage = pqstg.tile([128, NQ], F8, tag="qstg", name=f"qstg{t}")
            for tb in range(2):
                kq_chain(wp, "q", t, tb, stage)
            kq_folds("q", t, stage)

        def v_chain(wp, panel, tt):
            # V proj tile [128 tok, 512 w-cols] -> VO[:, tt, 8p:8p+8, 0:64]
            ps = ppj.tile([128, 512], F32, tag="ps")
            for ee in range(ET):
                nc.tensor.matmul(
                    ps,
                    lhsT=xT[:, ee, tt * 128 : (tt + 1) * 128],
                    rhs=wp[:, ee, :],
                    start=(ee == 0),
                    stop=(ee == ET - 1),
                )
            nc.vector.tensor_copy(
                VO[:, tt, 8 * panel : 8 * panel + 8, 0:64],
                ps.rearrange("p (h d) -> p h d", d=64),
            )

        # --- out-proj ---------------------------------------------------
        wob = [None, None]

        def boB_setup():
            # attn-out = AV/den + bv  (V-bias passes softmax unchanged), so
            # out = A_nobias @ Wo + (bv @ Wo + bo) = A_nobias @ Wo + boB
            ones128 = pc.tile([128, 128], BF16, tag="ones128")
            nc.vector.memset(ones128, 1.0)
            bv_rep = pc.tile([128, ET, 128], BF16, tag="bvrep")
            for ee in range(ET):
                nc.vector.tensor_scalar_mul(
                    bv_rep[:, ee, :], ones128, bqkvT[:, 16 + ee : 17 + ee]
                )
            for half in range(2):
                c0 = half * 512
                psb = ppj.tile([128, 512], F32, tag="ps")
                for ee in range(ET):
                    nc.tensor.matmul(
                        psb,
                        lhsT=bv_rep[:, ee, :],
                        rhs=wob[half][:, ee, :],
                        start=(ee == 0),
                        stop=(ee == ET - 1),
                    )
                nc.vector.tensor_add(boB[:, c0 : c0 + 512], psb, boT[:, c0 : c0 + 512])

        # three-stage out-proj: pass A (e-chunks 0:6 = pairs 0-5, windows
        # 12-14) -> bf16 partials incl. boB; pass B1 (chunk 6, windows
        # 14-15) adds pair 6; pass B2 (chunk 7) is the only tail work.
        NEA = 6
        partial = pprt.tile([128, 16, 512], BF16, tag="partial")

        def outproj_passA(tt, half):
            c0 = half * 512
            ps = ppj.tile([128, 512], F32, tag="ps")
            for ee in range(NEA):
                nc.tensor.matmul(
                    ps,
                    lhsT=AT[ee][:, tt * 128 : (tt + 1) * 128],
                    rhs=wob[half][:, ee, :],
                    start=(ee == 0),
                    stop=(ee == NEA - 1),
                )
            nc.vector.tensor_add(
                partial[:, tt * 2 + half, :], ps, boB[:, c0 : c0 + 512]
            )

        def outproj_passB1(tt, half):
            ps = ppj.tile([128, 512], F32, tag="ps")
            nc.tensor.matmul(
                ps,
                lhsT=AT[NEA][:, tt * 128 : (tt + 1) * 128],
                rhs=wob[half][:, NEA, :],
                start=True,
                stop=True,
            )
            nc.vector.scalar_tensor_tensor(
                partial[:, tt * 2 + half, :], ps, 1.0,
                partial[:, tt * 2 + half, :],
                op0=mybir.AluOpType.mult, op1=mybir.AluOpType.add,
            )

        def outproj_passB2(tt, half):
            c0 = half * 512
            ps = ppj.tile([128, 512], F32, tag="ps")
            nc.tensor.matmul(
                ps,
                lhsT=AT[7][:, tt * 128 : (tt + 1) * 128],
                rhs=wob[half][:, 7, :],
                start=True,
                stop=True,
            )
            osb = posb.tile([128, 512], F32, tag="osb")
            nc.vector.tensor_add(osb, ps, partial[:, tt * 2 + half, :])
            nc.sync.dma_start(
                out=out[tt * 128 : (tt + 1) * 128, c0 : c0 + 512], in_=osb
            )

        # --- prologue ---------------------------------------------------
        # PE warmup: keep the tensor engine busy from t~1us so the p-state
        # ramp completes before the first real chain arrives.
        warm = pc.tile([128, 512], BF16, tag="warm")
        nc.vector.memset(warm, 0.001)
        ps_w = pps.tile([128, NQ], F32, tag="ps_s", name="warmps")
        for i in range(26):
            nc.tensor.matmul(
                ps_w[:, 0:512], lhsT=warm[:, 0:128], rhs=warm,
                start=True, stop=True, skip_group_check=True,
            )

        ws_q0 = load_skinny(0)
        ws_k0 = load_skinny(E)
        for tt in range(4):
            load_x(tt)
        wp_k0 = load_panel(E)
        wp_q0 = load_panel(0)
        for tt in range(4, 16):
            load_x(tt)
        wp_v0 = load_panel(2 * E)
        wp_v1_pro = load_panel(2 * E + 512)

        # pair-tile 0 with per-tb folds, Q before K so full-Q is ready first
        stg_q0 = pqstg.tile([128, NQ], F8, tag="qstg", name="qstg0")
        stg_k0 = pkstg.tile([128, SEQ], F8, tag="kstg", name="kstg0")
        for tb in range(2):
            kq_chain(ws_q0, "q", 0, tb, stg_q0, skinny=True)
            kq_folds("q", 0, stg_q0, tb=tb)
        for tb in range(4):
            kq_chain(ws_k0, "k", 0, tb, stg_k0, skinny=True)
            kq_folds("k", 0, stg_k0, tb=tb)

        # --- deferred proj work, paced one job per kt slot ---------------
        wp_box = {}

        def defer_panel(name, pc0, src=None):
            def job():
                wp_box[name] = load_panel(pc0, src_w=src)

            return job

        def kq_jobs(kind, wp_name, t):
            # one chain per job; folds ride with the last chain
            ntb = 4 if kind == "k" else 2
            stage_box = {}

            def mk(tb):
                def job():
                    if tb == 0:
                        pool, shape = (pkstg, SEQ) if kind == "k" else (pqstg, NQ)
                        stage_box[0] = pool.tile(
                            [128, shape], F8, tag="kstg" if kind == "k" else "qstg",
                            name=f"{kind}stg{t}",
                        )
                    wp = wp_box[wp_name] if isinstance(wp_name, str) else wp_name
                    kq_chain(wp, kind, t, tb, stage_box[0])
                    if tb == ntb - 1:
                        kq_folds(kind, t, stage_box[0])

                return job

            return [mk(tb) for tb in range(ntb)]

        v1j = [(lambda t=t: v_chain(wp_v1_pro, 1, t)) for t in range(16)]
        jobs = {
            1: kq_jobs("k", wp_k0, 1) + kq_jobs("q", wp_q0, 1),
            2: v1j[0:5] + [defer_panel("k1", E + 512)],
            3: kq_jobs("k", wp_k0, 2) + kq_jobs("q", wp_q0, 2),
            4: v1j[5:10] + [defer_panel("q1", 512)],
            5: kq_jobs("k", wp_k0, 3) + kq_jobs("q", wp_q0, 3),
            6: v1j[10:16],
            7: kq_jobs("k", "k1", 4) + kq_jobs("q", "q1", 4),
            8: kq_jobs("k", "k1", 5) + kq_jobs("q", "q1", 5)
               + [defer_panel("wo0", 0, wo), defer_panel("wo1", 512, wo)],
            9: kq_jobs("k", "k1", 6) + kq_jobs("q", "q1", 6),
            10: kq_jobs("k", "k1", 7) + kq_jobs("q", "q1", 7),
            11: [lambda: (wob.__setitem__(0, wp_box["wo0"]),
                          wob.__setitem__(1, wp_box["wo1"])),
                 boB_setup],
            12: [(lambda t=t: outproj_passA(t, 0)) for t in range(6)],
            13: [(lambda t=t: outproj_passA(t, 1)) for t in range(6)]
               + [(lambda t=t: outproj_passA(t, 0)) for t in range(6, 8)],
            14: [(lambda t=t: outproj_passA(t, 1)) for t in range(6, 8)]
               + [(lambda t=t: outproj_passB1(t, 0)) for t in range(8)],
            15: [(lambda t=t: outproj_passB1(t, 1)) for t in range(8)],
        }

        # --- attention --------------------------------------------------
        cur_asb = [None] * QT

        def attention_head(h, inner=None):
            g, a = h // 4, h % 4
            p0 = 32 * a
            av = [
                pav.tile([128, 4, 65], F32, tag="av", name=f"av{h}_{i}")
                for i in range(2)
            ]
            for kt in range(KT):
                if inner is not None and kt < len(inner):
                    inner[kt]()
                ps_s = pps.tile([128, NQ], F32, tag="ps_s")
                for qh in range(2):
                    nc.tensor.matmul(
                        ps_s[:, qh * 512 : (qh + 1) * 512],
                        lhsT=K8[g][p0 : p0 + 32, :, kt * 128 : (kt + 1) * 128],
                        rhs=Q8[g][p0 : p0 + 32, :, qh * 512 : (qh + 1) * 512],
                        start=True,
                        stop=True,
                        perf_mode=DR,
                        tile_position=(p0, 0),
                    )
                pt = ppt.tile([128, NQ], BF16, tag="pt")
                nc.scalar.activation(pt, ps_s, AF.Exp, scale=0.125)
                for qt in range(QT):
                    nc.tensor.matmul(
                        av[qt // 4][:, qt % 4, :],
                        lhsT=pt[:, qt * 128 : (qt + 1) * 128],
                        rhs=VO[:, kt, h, :],
                        start=(kt == 0 and qt % 4 == 0),
                        stop=(kt == KT - 1),
                        skip_group_check=True,
                    )
            # normalize into a_sb (col half h%2); DMA-transpose pairs into AT
            hp = h // 2
            for qt in range(QT):
                if h % 2 == 0:
                    cur_asb[qt] = pasb.tile(
                        [128, 128], BF16, tag="asb", name=f"asb{h}_{qt}"
                    )
                rec = prec.tile([128, 1], F32, tag="rec", name=f"rec{h}_{qt}")
                nc.vector.reciprocal_approx_fast(
                    rec, av[qt // 4][:, qt % 4, 64:65]
                )
                nc.vector.tensor_scalar_mul(
                    cur_asb[qt][:, 64 * (h % 2) : 64 * (h % 2) + 64],
                    av[qt // 4][:, qt % 4, 0:64],
                    rec,
                )
                if h % 2 == 1:
                    nc.sync.dma_start(
                        out=AT[hp][:, qt * 128 : (qt + 1) * 128],
                        in_=cur_asb[qt],
                        transpose=True,
                    )

        inner0 = [(lambda t=tt: v_chain(wp_v0, 0, t)) for tt in range(TT)]
        for h in range(H):
            if h == 0:
                attention_head(h, inner=inner0)
                continue
            window_jobs = jobs.get(h, [])
            inner = [(lambda j=j: j()) for j in window_jobs]
            attention_head(h, inner=inner)

        # --- out-proj tail (pass B2 only) -------------------------------
        for tt in range(8):
            for half in range(2):
                outproj_passB2(tt, half)


_NC = None


def _get_program():
    global _NC
    if _NC is None:
        _NC = build_program()
    return _NC


def make_in_maps(x, Wqkv, bqkv, Wo, bo):
    w = {
        "Wqkv": np.ascontiguousarray(np.asarray(Wqkv, np.float32)),
        "bqkv": np.ascontiguousarray(np.asarray(bqkv, np.float32)),
        "Wo": np.ascontiguousarray(np.asarray(Wo, np.float32)),
        "bo": np.ascontiguousarray(np.asarray(bo, np.float32)),
    }
    x = np.asarray(x, np.float32)
    in_maps = []
    for c in range(N_CORES):
        b, s = divmod(c, 2)
        xb = x[b]
        if s == 1:
            xb = np.roll(xb, -NQ, axis=0)
        in_maps.append({"x": np.ascontiguousarray(xb), **w})
    return in_maps


def gather_out(results):
    out = np.empty((4, SEQ, E), np.float32)
    for c in range(N_CORES):
        b, s = divmod(c, 2)
        out[b, s * NQ : (s + 1) * NQ] = results[c]["out"]
    return out


def kernel(x, Wqkv, bqkv, Wo, bo):
    from concourse.bass_utils import run_bass_kernel_spmd

    nc = _get_program()
    in_maps = make_in_maps(x, Wqkv, bqkv, Wo, bo)
    res = run_bass_kernel_spmd(nc, in_maps, core_ids=list(range(N_CORES)))
    return gather_out(res.results)
